# revision 1
# baseline (speedup 1.0000x reference)
"""Trainium2 Bass kernel for nn_DrugRank (GNN message passing), 8 NeuronCores.

Strategy (SPMD, one program on 8 cores):
  - bio graph (50000 nodes, 800000 edges): destination-node sharding, 6250
    nodes/core. Edges bucketed by dst core, dst-sorted, grouped into 49
    blocks of 128 dst slots. Per block: bulk dma_gather of source-node
    feature rows from an AllGathered fp16 table, then a one-hot selection
    matmul accumulates the segment sum in PSUM (scatter via TensorE).
    Degrees are computed the same way (one-hot colsum matmul). The symmetric
    norm is folded as g = dinv*h into the gathered table, so only local dinv
    is ever needed.
  - cll graph (3451 nodes, padded 4096): same scheme, 512 nodes/core,
    4 blocks. Layer 4 (200->3) aggregates first, then transforms.
    The 10353x1000 dense is row-sharded (aligned with node sharding);
    partials summed with the single AllReduce that also broadcasts the
    bio branch's last-node row.
  - mol branch + fusion head: replicated on all cores (tiny).

dma_gather uses int16 indices, so the 50000-row bio table is addressed as
a lo half [0:32768] and hi half [32768:50000]; each block's edge list is
split into lo/hi segments (host-side index prep only).
"""

import numpy as np

import concourse.bacc as bacc
import concourse.bass as bass
import concourse.mybir as mybir
import concourse.tile as tile
from concourse._compat import cdiv
from concourse.bass_utils import run_bass_kernel_spmd

NCORES = 8
F = 200          # GCN feature width
FP = 256         # padded table width (512B fp16 rows for dma_gather)
P = 128

BIO_N, BIO_E, BIO_NPC = 50000, 800000, 6250
BIO_NBLK = cdiv(BIO_NPC, P)            # 49 (last block 106 slots)
BIO_LO = 32768                          # int16 index split
CLL_N, CLL_E, CLL_NPAD, CLL_NPC = 3451, 55216, 4096, 512
CLL_NBLK = CLL_NPC // P                 # 4
MOL_N, MOL_E = 64, 128

f32 = mybir.dt.float32
f16 = mybir.dt.float16
i16 = mybir.dt.int16
RELU = mybir.ActivationFunctionType.Relu
SQRT = mybir.ActivationFunctionType.Sqrt
ABS = mybir.ActivationFunctionType.Abs
COPY = mybir.ActivationFunctionType.Copy
EQ = mybir.AluOpType.is_equal
MUL = mybir.AluOpType.mult
ADD = mybir.AluOpType.add


# ---------------------------------------------------------------- host prep

def _pack_idx16(flat):
    """Pack int array (len = multiple of 128) into dma_gather idx layout:
    [128, len/16] int16; idx i at partition i%16, col i//16, tiled x8."""
    n = len(flat)
    a16 = np.asarray(flat, np.int16).reshape(n // 16, 16).T  # [16, n/16]
    return np.ascontiguousarray(np.tile(a16, (8, 1)))


def _pack_slots(flat, dtype=np.float16):
    """[128, len/128]; edge i at partition i%128, col i//128."""
    n = len(flat)
    return np.ascontiguousarray(
        np.asarray(flat, np.float64).astype(dtype).reshape(n // P, P).T)


def _prep_edges(edge, npc, nblk, lo_split):
    """Bucket edges by dst core, dst-sort, block-group, lo/hi split.

    Returns per-core packed (idx, slot) arrays plus per-block tile counts
    tlo[b], thi[b] (shared across cores = max, for the SPMD program)."""
    src = edge[0].astype(np.int64)
    dst = edge[1].astype(np.int64)
    per_core = []
    degs = []
    for c in range(NCORES):
        sel = (dst >= c * npc) & (dst < (c + 1) * npc)
        s, d = src[sel], dst[sel] - c * npc
        order = np.argsort(d, kind="stable")
        s, d = s[order], d[order]
        blocks = []
        for b in range(nblk):
            m = (d >= b * P) & (d < (b + 1) * P)
            sb, db = s[m], d[m] - b * P
            if lo_split:
                lo = sb < BIO_LO
                blocks.append((sb[lo], db[lo], sb[~lo] - BIO_LO, db[~lo]))
            else:
                blocks.append((sb, db, sb[:0], db[:0]))
        per_core.append(blocks)
        degs.append(np.bincount(d, minlength=nblk * P).astype(np.float64))
    tlo = [max(max(cdiv(len(per_core[c][b][0]), P), 1) for c in range(NCORES))
           for b in range(nblk)]
    thi = [max(cdiv(len(per_core[c][b][2]), P) for c in range(NCORES))
           for b in range(nblk)]
    idxs, slots = [], []
    for c in range(NCORES):
        fi, fs = [], []
        for b in range(nblk):
            slo, dlo, shi, dhi = per_core[c][b]
            for seg_s, seg_d, t in ((slo, dlo, tlo[b]), (shi, dhi, thi[b])):
                n = t * P
                pi = np.zeros(n, np.int64)
                ps = np.full(n, -1.0, np.float64)
                pi[:len(seg_s)] = seg_s
                ps[:len(seg_d)] = seg_d
                fi.append(pi)
                fs.append(ps)
        fi, fs = np.concatenate(fi), np.concatenate(fs)
        idxs.append(_pack_idx16(fi))
        slots.append(_pack_slots(fs))
    deg_tiles = [np.ascontiguousarray(d.reshape(nblk, P).T.astype(np.float32))
                 for d in degs]
    return idxs, slots, tlo, thi, deg_tiles


def _col(v):
    return np.ascontiguousarray(np.asarray(v, np.float32).reshape(-1, 1))


def _rep(v, rows=P):
    return np.ascontiguousarray(
        np.tile(np.asarray(v, np.float32).reshape(1, -1), (rows, 1)))


def _btile(v, p, n):
    """bias [p*n] -> [p, n] with column j = v[j*p:(j+1)*p]."""
    return np.ascontiguousarray(
        np.asarray(v, np.float32).reshape(n, p).T)


def prep_inputs(inp):
    """Build per-core in_maps + compile-time meta from the full inputs."""
    meta = {}
    bio_idx, bio_slot, meta["btlo"], meta["bthi"], bio_deg = _prep_edges(
        inp["edge_bio"], BIO_NPC, BIO_NBLK, lo_split=True)
    cll_idx, cll_slot, meta["ctlo"], cthi, cll_deg = _prep_edges(
        inp["edge_cll"], CLL_NPC, CLL_NBLK, lo_split=False)
    assert all(t == 0 for t in cthi)

    mol_s = inp["edge_mol"][0].astype(np.int64)
    mol_d = inp["edge_mol"][1].astype(np.int64)
    order = np.argsort(mol_d, kind="stable")
    mol_idx = _pack_idx16(mol_s[order])
    mol_slot = _pack_slots(mol_d[order].astype(np.float64), np.float32)

    xbT = np.ascontiguousarray(inp["x_bio"].T.astype(np.float32))   # [256, 50000]
    xcT = np.zeros((512, CLL_NPAD), np.float32)
    xcT[:, :CLL_N] = inp["x_cll"].T

    # per-channel row-sliced + zero-padded Wl1c: flat idx = node*3 + ch
    w1c = np.asarray(inp["Wl1c"], np.float32)                      # [3*3451, 1000]
    w1c_ch = np.zeros((3, CLL_NPAD, 1000), np.float32)
    for ch in range(3):
        w1c_ch[ch, :CLL_N] = w1c[ch::3]

    iota = np.tile(np.arange(P, dtype=np.float32), (P, 1))
    ident = np.eye(P, dtype=np.float32)
    ones_col = np.ones((P, 1), np.float32)

    shared = {
        "Wb1": np.asarray(inp["Wb1"], np.float16),
        "Wb2": np.asarray(inp["Wb2"], np.float16),
        "bb1_rep": _rep(inp["bb1"]), "bb2_rep": _rep(inp["bb2"]),
        "Wc1": np.asarray(inp["Wc1"], np.float16),
        "Wc2": np.asarray(inp["Wc2"], np.float16),
        "Wc3": np.asarray(inp["Wc3"], np.float16),
        "Wc4": np.asarray(inp["Wc4"], np.float16),
        "bc1_rep": _rep(inp["bc1"]), "bc2_rep": _rep(inp["bc2"]),
        "bc3_rep": _rep(inp["bc3"]), "bc4_rep": _rep(inp["bc4"]),
        "x_mol": np.asarray(inp["x_mol"], np.float32),
        "xmolT": np.ascontiguousarray(inp["x_mol"].T.astype(np.float32)),
        "mol_idx": mol_idx, "mol_slot": mol_slot,
        "Wm1r": np.asarray(inp["Wm1r"], np.float32),
        "Wm1s": np.asarray(inp["Wm1s"], np.float32),
        "Wm2r": np.asarray(inp["Wm2r"], np.float32),
        "Wm2s": np.asarray(inp["Wm2s"], np.float32),
        "bm1_rep": _rep(inp["bm1"]), "bm2_rep": _rep(inp["bm2"]),
        "Wlm": np.asarray(inp["Wlm"], np.float32), "blm_col": _col(inp["blm"]),
        "Wlb": np.asarray(inp["Wlb"], np.float32), "blb_col": _col(inp["blb"]),
        "Wd1": np.asarray(inp["Wd1"], np.float32), "bd1_t": _btile(inp["bd1"], 125, 4),
        "Wd2": np.asarray(inp["Wd2"], np.float32), "bd2_t": _btile(inp["bd2"], 128, 2),
        "Wcat1": np.asarray(inp["Wcat1"], np.float32),
        "bcat1_t": _btile(inp["bcat1"], 125, 8),
        "Wcat2": np.asarray(inp["Wcat2"], np.float32),
        "bcat2_t": np.asarray(inp["bcat2"], np.float32).reshape(1, 1),
        "bl1c_t": _btile(inp["bl1c"], 125, 8),
        "Wl2c": np.asarray(inp["Wl2c"], np.float32),
        "bl2c_t": _btile(inp["bl2c"], 125, 8),
        "Wl3c": np.asarray(inp["Wl3c"], np.float32),
        "bl3c_t": _btile(inp["bl3c"], 128, 2),
        "iota32": iota, "iota16": iota.astype(np.float16),
        "ident32": ident, "ident16": ident.astype(np.float16),
        "ones16": ones_col.astype(np.float16), "ones32": ones_col,
        "e105": np.eye(128, 1, k=-105, dtype=np.float32),
    }
    in_maps = []
    for c in range(NCORES):
        m = dict(shared)
        m["xbioT"] = np.ascontiguousarray(xbT[:, c * BIO_NPC:(c + 1) * BIO_NPC]).astype(np.float16)
        m["bio_idx"] = bio_idx[c]
        m["bio_slot"] = bio_slot[c]
        m["bio_deg"] = bio_deg[c]
        m["xcllT"] = np.ascontiguousarray(xcT[:, c * CLL_NPC:(c + 1) * CLL_NPC]).astype(np.float16)
        m["cll_idx"] = cll_idx[c]
        m["cll_slot"] = cll_slot[c]
        m["cll_deg"] = cll_deg[c]
        m["W1c_ch"] = np.ascontiguousarray(
            w1c_ch[:, c * CLL_NPC:(c + 1) * CLL_NPC, :])
        m["mask"] = np.full((1, 1), 1.0 if c == NCORES - 1 else 0.0, np.float32)
        in_maps.append(m)
    return in_maps, meta


# ------------------------------------------------------------ device program

RG = [list(range(NCORES))]


def _declare_inputs(nc, meta):
    tb = sum(meta["btlo"]) + sum(meta["bthi"])
    tc_ = sum(meta["ctlo"])
    spec = {
        "xbioT": ([256, BIO_NPC], f16),
        "bio_idx": ([P, 8 * tb], i16), "bio_slot": ([P, tb], f16),
        "bio_deg": ([P, BIO_NBLK], f32),
        "Wb1": ([256, F], f16), "Wb2": ([F, F], f16),
        "bb1_rep": ([P, F], f32), "bb2_rep": ([P, F], f32),
        "mask": ([1, 1], f32),
        "xcllT": ([512, CLL_NPC], f16),
        "cll_idx": ([P, 8 * tc_], i16), "cll_slot": ([P, tc_], f16),
        "cll_deg": ([P, CLL_NBLK], f32),
        "Wc1": ([512, F], f16), "Wc2": ([F, F], f16), "Wc3": ([F, F], f16),
        "Wc4": ([F, 3], f16),
        "bc1_rep": ([P, F], f32), "bc2_rep": ([P, F], f32),
        "bc3_rep": ([P, F], f32), "bc4_rep": ([P, 3], f32),
        "W1c_ch": ([3, CLL_NPC, 1000], f32),
        "x_mol": ([MOL_N, 64], f32), "xmolT": ([64, MOL_N], f32),
        "mol_idx": ([P, 8], i16), "mol_slot": ([P, 1], f32),
        "Wm1r": ([64, F], f32), "Wm1s": ([64, F], f32),
        "Wm2r": ([F, F], f32), "Wm2s": ([F, F], f32),
        "bm1_rep": ([P, F], f32), "bm2_rep": ([P, F], f32),
        "Wlm": ([F, 128], f32), "blm_col": ([128, 1], f32),
        "Wlb": ([F, 128], f32), "blb_col": ([128, 1], f32),
        "Wd1": ([256, 500], f32), "bd1_t": ([125, 4], f32),
        "Wd2": ([500, 256], f32), "bd2_t": ([128, 2], f32),
        "Wcat1": ([512, 1000], f32), "bcat1_t": ([125, 8], f32),
        "Wcat2": ([1000, 1], f32), "bcat2_t": ([1, 1], f32),
        "bl1c_t": ([125, 8], f32),
        "Wl2c": ([1000, 1000], f32), "bl2c_t": ([125, 8], f32),
        "Wl3c": ([1000, 256], f32), "bl3c_t": ([128, 2], f32),
        "iota32": ([P, P], f32), "iota16": ([P, P], f16),
        "ident32": ([P, P], f32), "ident16": ([P, P], f16),
        "ones16": ([P, 1], f16), "ones32": ([P, 1], f32),
        "e105": ([P, 1], f32),
    }
    return {k: nc.dram_tensor(k, s, d, kind="ExternalInput")
            for k, (s, d) in spec.items()}


def build_program(meta, repeat=1, extra=None):
    nc = bacc.Bacc("TRN2", target_bir_lowering=False, debug=False,
                   enable_asserts=False, num_devices=NCORES,
                   num_swdge_queues=4)
    io = _declare_inputs(nc, meta)
    out = nc.dram_tensor("out", [1, 1], f32, kind="ExternalOutput")

    g1_slice = nc.dram_tensor("g1_slice", [BIO_NPC, FP], f16, kind="Internal")
    g1_full = nc.dram_tensor("g1_full", [BIO_N, FP], f16, kind="Internal",
                             addr_space="Shared")
    g2_slice = nc.dram_tensor("g2_slice", [BIO_NPC, FP], f16, kind="Internal")
    g2_full = nc.dram_tensor("g2_full", [BIO_N, FP], f16, kind="Internal",
                             addr_space="Shared")
    gc_slice = [nc.dram_tensor(f"gc{l}_slice", [CLL_NPC, FP], f16,
                               kind="Internal") for l in range(4)]
    gc_full = [nc.dram_tensor(f"gc{l}_full", [CLL_NPAD, FP], f16,
                              kind="Internal", addr_space="Shared")
               for l in range(4)]
    m1_dram = nc.dram_tensor("m1_dram", [MOL_N, FP], f32, kind="Internal")
    out2_dram = nc.dram_tensor("out2_dram", [BIO_NPC, F], f32, kind="Internal")
    ar_in = nc.dram_tensor("ar_in", [1200], f32, kind="Internal")
    ar_out = nc.dram_tensor("ar_out", [1200], f32, kind="Internal",
                            addr_space="Shared")

    with tile.TileContext(nc) as tc:
        for _ in range(repeat):
            _build(nc, tc, meta, io, out, g1_slice, g1_full, g2_slice,
                   g2_full, gc_slice, gc_full, m1_dram, ar_in, ar_out,
                   out2_dram, extra)
    nc.compile()
    return nc


def _build(nc, tc, meta, io, out, g1_slice, g1_full, g2_slice, g2_full,
           gc_slice, gc_full, m1_dram, ar_in, ar_out, out2_dram, extra=None):
    btlo, bthi, ctlo = meta["btlo"], meta["bthi"], meta["ctlo"]

    with (
        tc.tile_pool(name="const", bufs=1) as cp,
        tc.tile_pool(name="wp", bufs=1) as wp,
        tc.tile_pool(name="big", bufs=1) as bigp,
        tc.tile_pool(name="gs", bufs=2) as gsp,
        tc.tile_pool(name="ct", bufs=2) as ctp,
        tc.tile_pool(name="sb", bufs=3) as sb,
        tc.tile_pool(name="vp", bufs=3) as vp,
        tc.tile_pool(name="mp", bufs=2) as mp,
        tc.tile_pool(name="psE", bufs=3, space="PSUM") as psE,
        tc.tile_pool(name="psT", bufs=2, space="PSUM") as psT,
        tc.tile_pool(name="psM", bufs=2, space="PSUM") as psM,
    ):
        def load(pool, name, rows=None, cols=None, tag=None):
            src = io[name]
            r = rows if rows is not None else src.shape[0]
            c = cols if cols is not None else src.shape[1]
            t = pool.tile([r, c], src.dtype, tag=tag or name)
            nc.sync.dma_start(t[:], src[0:r, 0:c])
            return t

        iota16 = load(cp, "iota16")
        iota32 = load(cp, "iota32")
        ident16 = load(cp, "ident16")
        ident32 = load(cp, "ident32")
        mask_sb = load(cp, "mask")
        e105 = load(cp, "e105")


        qrr = [0]

        def next_q():
            qrr[0] = (qrr[0] + 1) % 4
            return qrr[0]

        def onehot_block(sl, tb):
            """Whole block one-hot [128 edges, tb, 128 slots] in one DVE op."""
            mblk = mp.tile([P, tb, P], f16, tag="M", name="mblk")
            nc.vector.tensor_tensor(
                mblk[:], iota16[:, None, 0:P].broadcast_to([P, tb, P]),
                sl[:, :, None].broadcast_to([P, tb, P]), op=EQ)
            return mblk

        # local dinv = 1/sqrt(deg+1) from host-counted degrees
        def mk_dinv(key, nblk, tag):
            degt = load(cp, key)
            d = cp.tile([P, nblk], f32, tag=tag, name=tag)
            nc.vector.tensor_scalar(d[:], degt[:], 1.0, None, op0=ADD)
            nc.vector.reciprocal(d[:], d[:])
            nc.scalar.activation(d[:], d[:], SQRT)
            return d

        dinv_b = mk_dinv("bio_deg", BIO_NBLK, "dinvb")
        dinv_c = mk_dinv("cll_deg", CLL_NBLK, "dinvc")

        # ---------------- shared phase-A / edge-pass helpers ----------------
        def phase_scale_store(ps, w, j, gself, dinv, slice_dram):
            nc.vector.tensor_scalar(gself[:w, j * F:(j + 1) * F], ps[:w],
                                    dinv[:w, j:j + 1], None, op0=MUL)
            nc.sync.dma_start(slice_dram[j * P:j * P + w, 0:F],
                              gself[:w, j * F:(j + 1) * F])

        def allgather(src, dst):
            nc.gpsimd.collective_compute(
                "AllGather", mybir.AluOpType.bypass, replica_groups=RG,
                ins=[src.ap()], outs=[dst.ap()])

        def edge_pass(nblk, tlo, thi, idx_dram, slot_dram,
                      lo_ap, hi_ap, gself, evict):
            tbase = 0
            for b in range(nblk):
                tl, th = tlo[b], thi[b]
                tb = tl + th
                sl = sb.tile([P, tb], f16, tag="slotb", name="sl")
                nc.sync.dma_start(sl[:], slot_dram[:, tbase:tbase + tb])
                ix = sb.tile([P, tb * 8], i16, tag="idxb", name="ix")
                nc.sync.dma_start(ix[:], idx_dram[:, tbase * 8:(tbase + tb) * 8])
                ps = psE.tile([P, F], f32, tag="eacc", space="PSUM")
                v = vp.tile([P, max(tb, 1), FP], f16, tag="v")
                for off in range(0, tl, 8):      # <=1024 idxs per gather
                    n = min(8, tl - off)
                    nc.gpsimd.dma_gather(
                        v[:, off:off + n, :], lo_ap,
                        ix[:, off * 8:(off + n) * 8], n * P, n * P, FP,
                        queue_num=next_q())
                for off in range(0, th, 8):
                    n = min(8, th - off)
                    nc.gpsimd.dma_gather(
                        v[:, tl + off:tl + off + n, :], hi_ap,
                        ix[:, (tl + off) * 8:(tl + off + n) * 8],
                        n * P, n * P, FP, queue_num=next_q())
                mblk = onehot_block(sl, tb)
                for t in range(tb):
                    nc.tensor.matmul(ps[:], mblk[:, t, :], v[:, t, 0:F],
                                     start=(t == 0), stop=False)
                nc.tensor.matmul(ps[:], ident16[:], gself[:, b * F:(b + 1) * F],
                                 start=(tb == 0), stop=True)
                evict(b, ps)
                tbase += tb

        def evict_common(b, ps, w, dinv, brep, out_sb):
            t2 = sb.tile([P, F], f32, tag="ev1")
            nc.vector.tensor_scalar(t2[:w], ps[:w], dinv[:w, b:b + 1],
                                    None, op0=MUL)
            t3 = sb.tile([P, F], f32, tag="ev2")
            nc.vector.tensor_tensor(t3[:w], t2[:w], brep[:w], op=ADD)
            nc.scalar.activation(out_sb[:w], t3[:w], RELU)

        def transpose_to(src_sb, w, dst0, dst1, b):
            pt = psT.tile([P, P], f32, tag="tp", space="PSUM")
            nc.tensor.transpose(pt[0:P, 0:w], src_sb[:w, 0:P],
                                ident32[:w, :w])
            nc.vector.tensor_copy(dst0[:, b * P:b * P + w], pt[0:P, 0:w])
            pt2 = psT.tile([P, P], f32, tag="tp", space="PSUM")
            nc.tensor.transpose(pt2[0:F - P, 0:w], src_sb[:w, P:F],
                                ident32[:w, :w])
            nc.vector.tensor_copy(dst1[0:F - P, b * P:b * P + w],
                                  pt2[0:F - P, 0:w])

        # ---------------- bio phase A layer 1 ----------------
        wb1a = load(wp, "Wb1", rows=128, tag="wb1a")
        wb1b = wp.tile([P, F], f16, tag="wb1b")
        nc.sync.dma_start(wb1b[:], io["Wb1"][128:256, :])
        xb0 = bigp.tile([P, BIO_NPC], f16, tag="kbig0")
        nc.sync.dma_start(xb0[:], io["xbioT"][0:128, :])
        xb1 = bigp.tile([P, BIO_NPC], f16, tag="kbig1")
        nc.sync.dma_start(xb1[:], io["xbioT"][128:256, :])
        gs1 = gsp.tile([P, BIO_NBLK * F], f16, tag="gself")
        nc.vector.memset(gs1[:, 48 * F:49 * F], 0.0)
        for j in range(BIO_NBLK):
            w = min(P, BIO_NPC - j * P)
            ps = psM.tile([P, F], f32, tag="misc", space="PSUM")
            nc.tensor.matmul(ps[:w], xb0[:, j * P:j * P + w], wb1a[:],
                             start=True, stop=False)
            nc.tensor.matmul(ps[:w], xb1[:, j * P:j * P + w], wb1b[:],
                             start=False, stop=True)
            phase_scale_store(ps, w, j, gs1, dinv_b, g1_slice)
        allgather(g1_slice, g1_full)

        # ---------------- cll phase A layer 1 + all 4 cll layers -------------
        wc1 = [wp.tile([P, F], f16, tag=f"wc1_{k}", name=f"wc1_{k}")
               for k in range(4)]
        for k in range(4):
            nc.sync.dma_start(wc1[k][:], io["Wc1"][k * P:(k + 1) * P, :])
        xc = [bigp.tile([P, CLL_NPC], f16, tag=f"xc{k}", name=f"xc{k}")
              for k in range(4)]
        for k in range(4):
            nc.sync.dma_start(xc[k][:], io["xcllT"][k * P:(k + 1) * P, :])
        gsc = gsp.tile([P, CLL_NBLK * F], f16, tag="gcself")
        for j in range(CLL_NBLK):
            ps = psM.tile([P, F], f32, tag="misc", space="PSUM")
            for k in range(4):
                nc.tensor.matmul(ps[:], xc[k][:, j * P:(j + 1) * P], wc1[k][:],
                                 start=(k == 0), stop=(k == 3))
            phase_scale_store(ps, P, j, gsc, dinv_c, gc_slice[0])
        allgather(gc_slice[0], gc_full[0])

        wc2a = load(wp, "Wc2", rows=128, tag="wc2a")
        wc2b = wp.tile([P, F], f16, tag="wc2b")
        nc.sync.dma_start(wc2b[0:72, :], io["Wc2"][128:200, :])
        wc3a = load(wp, "Wc3", rows=128, tag="wc3a")
        wc3b = wp.tile([P, F], f16, tag="wc3b")
        nc.sync.dma_start(wc3b[0:72, :], io["Wc3"][128:200, :])
        bc_rep = [load(wp, f"bc{l}_rep") for l in (1, 2, 3)]

        cT0 = cT1 = None
        gs4_hold = []
        for layer in range(3):          # cll GCN layers 1..3 edge+evict
            nT0 = nT1 = None
            if layer < 2:
                nT0 = ctp.tile([P, CLL_NPC], f16, tag="ccT0")
                nT1 = ctp.tile([P, CLL_NPC], f16, tag="ccT1")
            gsc_l = gsc if layer == 0 else None

            if layer > 0:
                wa, wb = (wc2a, wc2b) if layer == 1 else (wc3a, wc3b)
                gsc_l = gsp.tile([P, CLL_NBLK * F], f16, tag="gcself")
                for j in range(CLL_NBLK):
                    ps = psM.tile([P, F], f32, tag="misc", space="PSUM")
                    nc.tensor.matmul(ps[:], cT0[:, j * P:(j + 1) * P], wa[:],
                                     start=True, stop=False)
                    nc.tensor.matmul(ps[:], cT1[0:72, j * P:(j + 1) * P],
                                     wb[0:72, :], start=False, stop=True)
                    phase_scale_store(ps, P, j, gsc_l, dinv_c, gc_slice[layer])
                allgather(gc_slice[layer], gc_full[layer])

            gs4 = None
            if layer == 2:
                gs4 = gsp.tile([P, CLL_NBLK * F], f16, tag="gcself")
                gs4_hold.append(gs4)

            def evict_cll(b, ps, layer=layer, nT0=nT0, nT1=nT1, gs4=gs4):
                bg = sb.tile([P, F], f32, tag="ev3")
                evict_common(b, ps, P, dinv_c, bc_rep[layer], bg)
                if layer < 2:
                    transpose_to(bg, P, nT0, nT1, b)
                else:
                    nc.vector.tensor_scalar(gs4[:, b * F:(b + 1) * F], bg[:],
                                            dinv_c[:, b:b + 1], None, op0=MUL)
                    nc.sync.dma_start(gc_slice[3][b * P:(b + 1) * P, 0:F],
                                      gs4[:, b * F:(b + 1) * F])

            edge_pass(CLL_NBLK, ctlo, [0] * CLL_NBLK, io["cll_idx"],
                      io["cll_slot"],
                      gc_full[layer].ap(), None, gsc_l, evict_cll)
            cT0, cT1 = nT0, nT1
        allgather(gc_slice[3], gc_full[3])

        # cll layer 4: aggregate gc4, then transform by Wc4
        agT0 = ctp.tile([P, CLL_NPC], f16, tag="ccT0")
        agT1 = ctp.tile([P, CLL_NPC], f16, tag="ccT1")

        def evict_cll4(b, ps):
            ag = sb.tile([P, F], f32, tag="ev3")
            nc.vector.tensor_scalar(ag[:], ps[:], dinv_c[:, b:b + 1],
                                    None, op0=MUL)
            transpose_to(ag, P, agT0, agT1, b)

        edge_pass(CLL_NBLK, ctlo, [0] * CLL_NBLK, io["cll_idx"],
                  io["cll_slot"],
                  gc_full[3].ap(), None, gs4_hold[0], evict_cll4)

        # ---------------- cll layer 4 transform: h4 = relu(agg4@Wc4+bc4) -----
        wc4a = load(wp, "Wc4", rows=128, tag="wc4a")
        wc4b = wp.tile([P, 3], f16, tag="wc4b")
        nc.sync.dma_start(wc4b[0:72, :], io["Wc4"][128:200, :])
        bc4r = load(wp, "bc4_rep")
        h4_all = sb.tile([P, 12], f32, tag="h4")
        for j in range(CLL_NBLK):
            psh = psM.tile([P, 3], f32, tag="misc", space="PSUM")
            nc.tensor.matmul(psh[:], agT0[:, j * P:(j + 1) * P], wc4a[:],
                             start=True, stop=False)
            nc.tensor.matmul(psh[:], agT1[0:72, j * P:(j + 1) * P],
                             wc4b[0:72, :], start=False, stop=True)
            th = sb.tile([P, 3], f32, tag="th4")
            nc.vector.tensor_tensor(th[:], psh[:], bc4r[:, 0:3], op=ADD)
            nc.scalar.activation(h4_all[:, j * 3:(j + 1) * 3], th[:], RELU)

        # ---------------- bio edge layer 1 ----------------
        bb1r = load(wp, "bb1_rep")
        bb2r = load(wp, "bb2_rep")
        bgr1T0 = bigp.tile([P, BIO_NBLK * P], f16, tag="kbig0")
        bgr1T1 = bigp.tile([P, BIO_NBLK * P], f16, tag="kbig1")

        def evict_b1(b, ps):
            w = min(P, BIO_NPC - b * P)
            bg = sb.tile([P, F], f32, tag="ev3")
            evict_common(b, ps, w, dinv_b, bb1r, bg)
            transpose_to(bg, w, bgr1T0, bgr1T1, b)

        edge_pass(BIO_NBLK, btlo, bthi, io["bio_idx"], io["bio_slot"],
                  g1_full.ap()[0:BIO_LO, :], g1_full.ap()[BIO_LO:BIO_N, :],
                  gs1, evict_b1)

        # ---------------- bio phase A layer 2 + AG ----------------
        wb2a = load(wp, "Wb2", rows=128, tag="wb2a")
        wb2b = wp.tile([P, F], f16, tag="wb2b")
        nc.sync.dma_start(wb2b[0:72, :], io["Wb2"][128:200, :])
        gs2 = gsp.tile([P, BIO_NBLK * F], f16, tag="gself")
        nc.vector.memset(gs2[:, 48 * F:49 * F], 0.0)
        for j in range(BIO_NBLK):
            w = min(P, BIO_NPC - j * P)
            ps = psM.tile([P, F], f32, tag="misc", space="PSUM")
            nc.tensor.matmul(ps[:w], bgr1T0[:, j * P:j * P + w], wb2a[:],
                             start=True, stop=False)
            nc.tensor.matmul(ps[:w], bgr1T1[0:72, j * P:j * P + w],
                             wb2b[0:72, :], start=False, stop=True)
            phase_scale_store(ps, w, j, gs2, dinv_b, g2_slice)
        allgather(g2_slice, g2_full)

        # ---------------- bio edge layer 2 + last-row extraction -------------
        def evict_b2(b, ps):
            w = min(P, BIO_NPC - b * P)
            bg = sb.tile([P, F], f32, tag="ev3")
            evict_common(b, ps, w, dinv_b, bb2r, bg)
            nc.sync.dma_start(out2_dram[b * P:b * P + w, 0:F], bg[:w, :])
            if b == BIO_NBLK - 1:
                rps = psM.tile([1, F], f32, tag="misc", space="PSUM")
                nc.tensor.matmul(rps[:], e105[0:w, :], bg[0:w, :],
                                 start=True, stop=True)
                mrow = sb.tile([1, F], f32, tag="mrow")
                nc.vector.tensor_scalar(mrow[0:1, :], rps[0:1, :],
                                        mask_sb[0:1, 0:1], None, op0=MUL)
                nc.sync.dma_start(ar_in[1000:1200], mrow[0:1, :])

        edge_pass(BIO_NBLK, btlo, bthi, io["bio_idx"], io["bio_slot"],
                  g2_full.ap()[0:BIO_LO, :], g2_full.ap()[BIO_LO:BIO_N, :],
                  gs2, evict_b2)

        # ---------------- cll dense-1 row-sharded partials ----------------
        # Each (ch, ktile) step runs 8 independent single matmuls into one
        # PSUM tile's columns, then DVE-accumulates into SBUF (avoids
        # overlapping PSUM accumulation groups in one bank).
        acc_d = sb.tile([125, 8], f32, tag="accd")
        steps = [(c, k) for c in range(3) for k in range(4)]
        for i, (ch, kt) in enumerate(steps):
            wt = sb.tile([P, 1000], f32, tag="whead", bufs=2)
            nc.sync.dma_start(wt[:], io["W1c_ch"][ch, kt * P:(kt + 1) * P, :])
            pst = psM.tile([125, 8], f32, tag="misc", space="PSUM")
            for och in range(8):
                nc.tensor.matmul(pst[:, och:och + 1],
                                 wt[:, och * 125:(och + 1) * 125],
                                 h4_all[:, kt * 3 + ch:kt * 3 + ch + 1],
                                 start=True, stop=True)
            if i == 0:
                nc.vector.tensor_copy(acc_d[:], pst[:])
            else:
                nc.vector.tensor_tensor(acc_d[:], acc_d[:], pst[:], op=ADD)
        for j in range(8):
            nc.sync.dma_start(ar_in.ap()[j * 125:(j + 1) * 125, None],
                              acc_d[:, j:j + 1])

        # ---------------- mol branch (replicated, tiny) ----------------
        mol_idx_sb = load(cp, "mol_idx")
        mol_slot_sb = load(cp, "mol_slot")
        xmolT_sb = load(wp, "xmolT")
        wm1r = load(wp, "Wm1r")
        wm1s = load(wp, "Wm1s")
        bm1r = load(wp, "bm1_rep")
        bm2r = load(wp, "bm2_rep")
        v1 = sb.tile([P, 1, 64], f32, tag="vm")
        nc.gpsimd.dma_gather(v1[:], io["x_mol"].ap(), mol_idx_sb[:],
                             MOL_E, MOL_E, 64)
        mM = mp.tile([P, 64], f32, tag="Mmol")
        nc.vector.tensor_scalar(mM[:], iota32[:, 0:64], mol_slot_sb[:, 0:1],
                                None, op0=EQ)
        agg_ps = psM.tile([64, 64], f32, tag="misc", space="PSUM")
        nc.tensor.matmul(agg_ps[:], mM[:], v1[:, 0, :], start=True, stop=True)
        agg_sb = sb.tile([64, 64], f32, tag="mol1")
        nc.vector.tensor_copy(agg_sb[:], agg_ps[:])
        pt = psT.tile([P, P], f32, tag="tp", space="PSUM")
        nc.tensor.transpose(pt[0:64, 0:64], agg_sb[0:64, 0:64],
                            ident32[0:64, 0:64])
        aggT = sb.tile([64, 64], f32, tag="mol2")
        nc.vector.tensor_copy(aggT[:], pt[0:64, 0:64])
        h1_ps = psM.tile([64, F], f32, tag="misc", space="PSUM")
        nc.tensor.matmul(h1_ps[:], aggT[:], wm1r[:], start=True, stop=False)
        nc.tensor.matmul(h1_ps[:], xmolT_sb[:], wm1s[:], start=False, stop=True)
        t_m1 = sb.tile([64, F], f32, tag="mol3")
        nc.vector.tensor_tensor(t_m1[:], h1_ps[:], bm1r[0:64, :], op=ADD)
        m1_sb = sb.tile([64, F], f32, tag="mol4")
        nc.scalar.activation(m1_sb[:], t_m1[:], RELU)
        nc.sync.dma_start(m1_dram[0:64, 0:F], m1_sb[:])

        wm2r0 = load(wp, "Wm2r", rows=128, tag="wm2r0")
        wm2r1 = wp.tile([P, F], f32, tag="wm2r1")
        nc.sync.dma_start(wm2r1[0:72, :], io["Wm2r"][128:200, :])
        wm2s0 = load(wp, "Wm2s", rows=128, tag="wm2s0")
        wm2s1 = wp.tile([P, F], f32, tag="wm2s1")
        nc.sync.dma_start(wm2s1[0:72, :], io["Wm2s"][128:200, :])
        v2 = sb.tile([P, 1, FP], f32, tag="vm2")
        nc.gpsimd.dma_gather(v2[:], m1_dram.ap(), mol_idx_sb[:],
                             MOL_E, MOL_E, FP)
        agg2_ps = psM.tile([64, F], f32, tag="misc", space="PSUM")
        nc.tensor.matmul(agg2_ps[:], mM[:], v2[:, 0, 0:F], start=True, stop=True)
        agg2_sb = sb.tile([64, F], f32, tag="mol1")
        nc.vector.tensor_copy(agg2_sb[:], agg2_ps[:])
        a2T0 = sb.tile([P, 64], f32, tag="mol5")
        a2T1 = sb.tile([P, 64], f32, tag="mol6")
        m1T0 = sb.tile([P, 64], f32, tag="mol7")
        m1T1 = sb.tile([P, 64], f32, tag="mol8")
        for srcT, d0, d1 in ((agg2_sb, a2T0, a2T1), (m1_sb, m1T0, m1T1)):
            pt1 = psT.tile([P, P], f32, tag="tp", space="PSUM")
            nc.tensor.transpose(pt1[0:P, 0:64], srcT[0:64, 0:P],
                                ident32[0:64, 0:64])
            nc.vector.tensor_copy(d0[:, 0:64], pt1[0:P, 0:64])
            pt2 = psT.tile([P, P], f32, tag="tp", space="PSUM")
            nc.tensor.transpose(pt2[0:72, 0:64], srcT[0:64, P:F],
                                ident32[0:64, 0:64])
            nc.vector.tensor_copy(d1[0:72, 0:64], pt2[0:72, 0:64])
        h2_ps = psM.tile([64, F], f32, tag="misc", space="PSUM")
        nc.tensor.matmul(h2_ps[:], a2T0[:, 0:64], wm2r0[:],
                         start=True, stop=False)
        nc.tensor.matmul(h2_ps[:], a2T1[0:72, 0:64], wm2r1[0:72, :],
                         start=False, stop=False)
        nc.tensor.matmul(h2_ps[:], m1T0[:, 0:64], wm2s0[:],
                         start=False, stop=False)
        nc.tensor.matmul(h2_ps[:], m1T1[0:72, 0:64], wm2s1[0:72, :],
                         start=False, stop=True)
        t_m2 = sb.tile([64, F], f32, tag="mol3")
        nc.vector.tensor_tensor(t_m2[:], h2_ps[:], bm2r[0:64, :], op=ADD)
        m2_sb = sb.tile([64, F], f32, tag="mol4")
        nc.scalar.activation(m2_sb[:], t_m2[:], RELU)

        ones32_sb = load(cp, "ones32")
        wlm0 = load(wp, "Wlm", rows=128, tag="wlm0")
        wlm1 = wp.tile([P, 128], f32, tag="wlm1")
        nc.sync.dma_start(wlm1[0:72, :], io["Wlm"][128:200, :])
        blm = load(wp, "blm_col")
        mcol0 = sb.tile([P, 1], f32, tag="mc0")
        mcol1 = sb.tile([P, 1], f32, tag="mc1")
        pool_ps = psM.tile([P, 1], f32, tag="misc", space="PSUM")
        nc.tensor.matmul(pool_ps[0:P, :], m2_sb[0:64, 0:P], ones32_sb[0:64, :],
                         start=True, stop=True)
        nc.scalar.activation(mcol0[:], pool_ps[0:P, :], COPY, scale=1.0 / 64.0)
        pool_ps2 = psM.tile([P, 1], f32, tag="misc", space="PSUM")
        nc.tensor.matmul(pool_ps2[0:72, :], m2_sb[0:64, P:F], ones32_sb[0:64, :],
                         start=True, stop=True)
        nc.scalar.activation(mcol1[0:72, :], pool_ps2[0:72, :], COPY,
                             scale=1.0 / 64.0)
        mvec = sb.tile([P, 1], f32, tag="mvec")
        mm_ps = psM.tile([P, 1], f32, tag="misc", space="PSUM")
        nc.tensor.matmul(mm_ps[:], wlm0[:], mcol0[:], start=True, stop=False)
        nc.tensor.matmul(mm_ps[:], wlm1[0:72, :], mcol1[0:72, :],
                         start=False, stop=True)
        nc.scalar.activation(mvec[:], mm_ps[:], RELU, bias=blm[:])

        # ---------------- AllReduce (dense partials + bio last row) ----------
        nc.gpsimd.collective_compute(
            "AllReduce", mybir.AluOpType.add, replica_groups=RG,
            ins=[ar_in.ap()], outs=[ar_out.ap()])

        # ---------------- fusion head (replicated) ----------------
        def mm_chain(p_rows, n_cols, k_steps, act_bias, out_tag):
            """acc = sum_k (lhsT_k.T @ rhs_k) per column; relu(acc+bias)."""
            acc = sb.tile([p_rows, n_cols], f32, tag=out_tag + "a")
            for k in range(k_steps):
                lhsT, rhs = yield k
                pst = psM.tile([p_rows, n_cols], f32, tag="misc", space="PSUM")
                for och in range(n_cols):
                    nc.tensor.matmul(pst[:, och:och + 1], lhsT(och), rhs,
                                     start=True, stop=True)
                if k == 0:
                    nc.vector.tensor_copy(acc[:], pst[:])
                else:
                    nc.vector.tensor_tensor(acc[:], acc[:], pst[:], op=ADD)
            o = sb.tile([p_rows, n_cols], f32, tag=out_tag)
            for och in range(n_cols):
                nc.scalar.activation(o[:, och:och + 1], acc[:, och:och + 1],
                                     RELU, bias=act_bias[:, och:och + 1])
            yield o

        def run_chain(p_rows, n_cols, pieces, act_bias, out_tag):
            """pieces: list of (lhsT_fn(och), rhs_ap)."""
            gen = mm_chain(p_rows, n_cols, len(pieces), act_bias, out_tag)
            k = next(gen)
            while True:
                r = gen.send(pieces[k])
                if not isinstance(r, int):
                    return r
                k = r

        wlb0 = load(wp, "Wlb", rows=128, tag="wlb0")
        wlb1 = wp.tile([P, 128], f32, tag="wlb1")
        nc.sync.dma_start(wlb1[0:72, :], io["Wlb"][128:200, :])
        blb = load(wp, "blb_col")
        bgc0 = sb.tile([P, 1], f32, tag="bgc0")
        nc.sync.dma_start(bgc0[:], ar_out.ap()[1000:1128, None])
        bgc1 = sb.tile([P, 1], f32, tag="bgc1")
        nc.sync.dma_start(bgc1[0:72, :], ar_out.ap()[1128:1200, None])
        bvec = run_chain(P, 1, [
            (lambda o: wlb0[:, 0:128], bgc0[:]),
            (lambda o: wlb1[0:72, 0:128], bgc1[0:72, :]),
        ], blb, "bvec")

        wd1_0 = load(wp, "Wd1", rows=128, tag="wd10")
        wd1_1 = wp.tile([P, 500], f32, tag="wd11")
        nc.sync.dma_start(wd1_1[:], io["Wd1"][128:256, :])
        bd1 = load(wp, "bd1_t")
        d1 = run_chain(125, 4, [
            (lambda o: wd1_0[:, o * 125:(o + 1) * 125], mvec[:]),
            (lambda o: wd1_1[:, o * 125:(o + 1) * 125], bvec[:]),
        ], bd1, "d1")

        wd2t = [wp.tile([125, 256], f32, tag=f"wd2_{k}", name=f"wd2_{k}")
                for k in range(4)]
        for k in range(4):
            nc.sync.dma_start(wd2t[k][:], io["Wd2"][k * 125:(k + 1) * 125, :])
        bd2 = load(wp, "bd2_t")
        d2 = run_chain(P, 2, [
            (lambda o, k=k: wd2t[k][:, o * P:(o + 1) * P], d1[:, k:k + 1])
            for k in range(4)
        ], bd2, "d2")

        bl1c = load(wp, "bl1c_t")
        c1 = sb.tile([125, 8], f32, tag="c1")
        for j in range(8):
            tmpc = sb.tile([125, 1], f32, tag="ctmp")
            nc.sync.dma_start(tmpc[:], ar_out.ap()[j * 125:(j + 1) * 125, None])
            nc.scalar.activation(c1[:, j:j + 1], tmpc[:], RELU,
                                 bias=bl1c[:, j:j + 1])

        bl2c = load(wp, "bl2c_t")
        wtc2 = []
        for k in range(8):
            wt = sb.tile([125, 1000], f32, tag="whead", bufs=2)
            nc.sync.dma_start(wt[:], io["Wl2c"][k * 125:(k + 1) * 125, :])
            wtc2.append(wt)
        c2 = run_chain(125, 8, [
            (lambda o, k=k: wtc2[k][:, o * 125:(o + 1) * 125], c1[:, k:k + 1])
            for k in range(8)
        ], bl2c, "c2")

        bl3c = load(wp, "bl3c_t")
        wtc3 = []
        for k in range(8):
            wt = sb.tile([125, 256], f32, tag="whead2", bufs=2)
            nc.sync.dma_start(wt[:], io["Wl3c"][k * 125:(k + 1) * 125, :])
            wtc3.append(wt)
        c3 = run_chain(P, 2, [
            (lambda o, k=k: wtc3[k][:, o * P:(o + 1) * P], c2[:, k:k + 1])
            for k in range(8)
        ], bl3c, "c3")

        bcat1 = load(wp, "bcat1_t")
        rhs_cat = [d2[:, 0:1], d2[:, 1:2], c3[:, 0:1], c3[:, 1:2]]
        wtu = []
        for k in range(4):
            wt = sb.tile([P, 1000], f32, tag="whead", bufs=2)
            nc.sync.dma_start(wt[:], io["Wcat1"][k * P:(k + 1) * P, :])
            wtu.append(wt)
        u = run_chain(125, 8, [
            (lambda o, k=k: wtu[k][:, o * 125:(o + 1) * 125], rhs_cat[k])
            for k in range(4)
        ], bcat1, "u")

        wcat2 = wp.tile([125, 8], f32, tag="wcat2")
        for k in range(8):
            nc.sync.dma_start(wcat2[:, k:k + 1],
                              io["Wcat2"][k * 125:(k + 1) * 125, 0:1])
        bcat2 = load(wp, "bcat2_t")
        pso = psM.tile([1, 1], f32, tag="misc", space="PSUM")
        for k in range(8):
            nc.tensor.matmul(pso[:], wcat2[:, k:k + 1], u[:, k:k + 1],
                             start=(k == 0), stop=(k == 7))
        osb = sb.tile([1, 1], f32, tag="osb")
        nc.scalar.activation(osb[:], pso[:], RELU, bias=bcat2[:])
        nc.sync.dma_start(out[0:1, 0:1], osb[:])

        # ---- slope-benchmark probes (never used by kernel()) ----
        if extra:
            kind, k = extra
            for rep_i in range(k):
                if kind == "ag":
                    nc.gpsimd.collective_compute(
                        "AllGather", mybir.AluOpType.bypass, replica_groups=RG,
                        ins=[g1_slice.ap()], outs=[g1_full.ap()])
                elif kind == "edge":
                    def ev(b, ps, ri=rep_i):
                        w = min(P, BIO_NPC - b * P)
                        bg = sb.tile([P, F], f32, tag="ev3", name="bgx")
                        evict_common(b, ps, w, dinv_b, bb1r, bg)
                        nc.sync.dma_start(out2_dram[b * P:b * P + w, 0:F],
                                          bg[:w, :])
                    edge_pass(BIO_NBLK, btlo, bthi, io["bio_idx"],
                              io["bio_slot"],
                              g1_full.ap()[0:BIO_LO, :],
                              g1_full.ap()[BIO_LO:BIO_N, :], gs1, ev)
                elif kind == "gather":
                    tbase = 0
                    for b in range(BIO_NBLK):
                        tl, th = btlo[b], bthi[b]
                        tb = tl + th
                        ix = sb.tile([P, tb * 8], i16, tag="idxb", name="ixx")
                        nc.sync.dma_start(
                            ix[:], io["bio_idx"][:, tbase * 8:(tbase + tb) * 8])
                        v = vp.tile([P, max(tb, 1), FP], f16, tag="v",
                                    name="vx")
                        for off in range(0, tl, 8):
                            n = min(8, tl - off)
                            nc.gpsimd.dma_gather(
                                v[:, off:off + n, :], g1_full.ap()[0:BIO_LO, :],
                                ix[:, off * 8:(off + n) * 8], n * P, n * P, FP,
                                queue_num=next_q())
                        for off in range(0, th, 8):
                            n = min(8, th - off)
                            nc.gpsimd.dma_gather(
                                v[:, tl + off:tl + off + n, :],
                                g1_full.ap()[BIO_LO:BIO_N, :],
                                ix[:, (tl + off) * 8:(tl + off + n) * 8],
                                n * P, n * P, FP, queue_num=next_q())
                        tbase += tb


# ------------------------------------------------------------------- entry

_CACHE = {}


def kernel(**inputs):
    in_maps, meta = prep_inputs(inputs)
    key = (tuple(meta["btlo"]), tuple(meta["bthi"]), tuple(meta["ctlo"]))
    if key not in _CACHE:
        _CACHE[key] = build_program(meta)
    nc = _CACHE[key]
    res = run_bass_kernel_spmd(nc, in_maps, core_ids=list(range(NCORES)))
    return np.asarray(res.results[0]["out"], np.float32)



# revision 6
# speedup vs baseline: 6.8551x; 6.8551x over previous
"""Trainium2 Bass kernel for nn_DrugRank (GNN message passing), 8 NeuronCores.

Architecture (v2 — dense-block aggregation, pruned bio branch):

  - The reference consumes only row -1 (node 49999) of the bio GCN stack, so
    the 800k-edge bio branch is dead code except the 2-hop in-neighborhood of
    that node: ~16 L2 edges and ~280 L1 edges. Host prep extracts that
    neighborhood; the device does a handful of matmuls, replicated per core.
  - cll graph (3451 nodes, 55216 edges, 4 GCN layers): dst-node sharded,
    512 nodes (4 blocks of 128) per core. All GCN normalization (symmetric
    deg^-1/2 + self loops) is folded host-side into dense per-(src-chunk,
    dst-block) adjacency tiles Q[s,d] = dinv_s*dinv_d*cnt(s->d) +
    dinv_d^2*[s==d]; aggregation = relu(Q^T h + b) via 27x4 PSUM-accumulated
    128x128x200 matmuls per layer. No dma_gather anywhere in the cll path;
    identical work on every core (no stragglers).
  - Per layer: transform own slice (h = c @ W via PE transposes), AllGather
    the 200KB slice, reload the 1.35MB table, aggregate. 4 AllGathers + one
    1000-float AllReduce (dense-1 partials) are the only collectives.
  - mol branch + fusion head replicated on every core; head weights stream
    during the cll chain. Dense-1 (10353x1000) row-sharded with the node
    sharding, reduced by the AllReduce.
"""

import numpy as np

import concourse.bacc as bacc
import concourse.bass as bass
import concourse.mybir as mybir
import concourse.tile as tile
from concourse.bass_utils import run_bass_kernel_spmd

NCORES = 8
P = 128
F = 200

CLL_N, CLL_E, CLL_PAD, CLL_NPC = 3451, 55216, 4096, 512
CLL_NBLK = CLL_NPC // P                 # 4 dst blocks / core
CLL_NCH = 27                            # src chunks with real nodes
N_BIO = 50000
BIO_S2, BIO_S1 = 768, 128               # padded bio 2-hop sets
BIO_NCH = BIO_S2 // P                   # 6
MOL_N, MOL_E = 64, 128

f32 = mybir.dt.float32
f16 = mybir.dt.float16
i16 = mybir.dt.int16
RELU = mybir.ActivationFunctionType.Relu
COPY = mybir.ActivationFunctionType.Copy
EQ = mybir.AluOpType.is_equal
MUL = mybir.AluOpType.mult
ADD = mybir.AluOpType.add


# ---------------------------------------------------------------- host prep

def _pack_idx16(flat):
    n = len(flat)
    a16 = np.asarray(flat, np.int16).reshape(n // 16, 16).T
    return np.ascontiguousarray(np.tile(a16, (8, 1)))


def _pack_slots(flat, dtype=np.float16):
    n = len(flat)
    return np.ascontiguousarray(
        np.asarray(flat, np.float64).astype(dtype).reshape(n // P, P).T)


def _col(v):
    return np.ascontiguousarray(np.asarray(v, np.float32).reshape(-1, 1))


def _rep(v, rows=P):
    return np.ascontiguousarray(
        np.tile(np.asarray(v, np.float32).reshape(1, -1), (rows, 1)))


def _btile(v, p, n):
    return np.ascontiguousarray(np.asarray(v, np.float32).reshape(n, p).T)


def _cll_q(edge, dinv):
    """Dense normalized adjacency, [CLL_PAD, CLL_PAD] f32."""
    src = edge[0].astype(np.int64)
    dst = edge[1].astype(np.int64)
    q = np.zeros((CLL_PAD, CLL_PAD), np.float32)
    np.add.at(q, (src, dst), (dinv[src] * dinv[dst]).astype(np.float32))
    di = np.arange(CLL_N)
    q[di, di] += (dinv[:CLL_N] * dinv[:CLL_N]).astype(np.float32)
    return q


def _bio_prune(edge, x_bio):
    """2-hop in-neighborhood of node N_BIO-1 -> (xbT_sel, Qb1_pk, Qb2)."""
    src = edge[0].astype(np.int64)
    dst = edge[1].astype(np.int64)
    deg = np.bincount(dst, minlength=N_BIO).astype(np.float64) + 1.0
    dinv = 1.0 / np.sqrt(deg)
    tgt = N_BIO - 1

    m2 = dst == tgt
    s1 = np.unique(np.concatenate([src[m2], [tgt]]))
    assert len(s1) <= BIO_S1, len(s1)
    pos1 = np.full(N_BIO, -1, np.int64)
    pos1[s1] = np.arange(len(s1))

    m1 = pos1[dst] >= 0
    e1s, e1d = src[m1], dst[m1]
    s2 = np.unique(np.concatenate([e1s, s1]))
    assert len(s2) <= BIO_S2, len(s2)
    pos2 = np.full(N_BIO, -1, np.int64)
    pos2[s2] = np.arange(len(s2))

    q1 = np.zeros((BIO_S2, BIO_S1), np.float32)
    np.add.at(q1, (pos2[e1s], pos1[e1d]),
              (dinv[e1s] * dinv[e1d]).astype(np.float32))
    q1[pos2[s1], pos1[s1]] += (dinv[s1] * dinv[s1]).astype(np.float32)

    q2 = np.zeros((BIO_S1, 1), np.float32)
    np.add.at(q2, (pos1[src[m2]], 0),
              (dinv[src[m2]] * dinv[tgt]).astype(np.float32))
    q2[pos1[tgt], 0] += np.float32(dinv[tgt] * dinv[tgt])

    xsel = np.zeros((BIO_S2, 256), np.float32)
    xsel[:len(s2)] = x_bio[s2]
    xbT = np.ascontiguousarray(xsel.T).astype(np.float16)      # [256, 768]
    # Qb1 packed [128, 6*128]: [p, s*128+d] = q1[s*128+p, d]
    q1pk = np.ascontiguousarray(
        q1.reshape(BIO_NCH, P, BIO_S1).transpose(1, 0, 2)
        .reshape(P, BIO_NCH * BIO_S1)).astype(np.float16)
    return xbT, q1pk, q2.astype(np.float16)


def prep_inputs(inp):
    meta = {}
    # ---- cll Q tiles ----
    dst = inp["edge_cll"][1].astype(np.int64)
    deg = np.bincount(dst, minlength=CLL_N).astype(np.float64) + 1.0
    dinv = np.zeros(CLL_PAD, np.float64)
    dinv[:CLL_N] = 1.0 / np.sqrt(deg)
    q = _cll_q(inp["edge_cll"], dinv)

    xcT = np.zeros((512, CLL_PAD), np.float32)
    xcT[:, :CLL_N] = inp["x_cll"].T

    # W1c regrouped: rows (node*3+ch) -> per core [128, 12*1000] f16,
    # col-block j = ch*4+blk, rows = local node p of that block.
    w1c = np.asarray(inp["Wl1c"], np.float32)                  # [10353, 1000]
    w1c_n = np.zeros((CLL_PAD, 3, 1000), np.float32)
    w1c_n[:CLL_N] = w1c.reshape(CLL_N, 3, 1000)

    xbT_sel, q1pk, q2 = _bio_prune(inp["edge_bio"], np.asarray(inp["x_bio"]))

    mol_s = inp["edge_mol"][0].astype(np.int64)
    mol_d = inp["edge_mol"][1].astype(np.int64)
    order = np.argsort(mol_d, kind="stable")
    mol_idx = _pack_idx16(mol_s[order])
    mol_slot = _pack_slots(mol_d[order].astype(np.float64), np.float32)

    iota = np.tile(np.arange(P, dtype=np.float32), (P, 1))
    ident = np.eye(P, dtype=np.float32)

    shared = {
        "Wc1": np.asarray(inp["Wc1"], np.float16),
        "Wc2": np.asarray(inp["Wc2"], np.float16),
        "Wc3": np.asarray(inp["Wc3"], np.float16),
        "Wc4": np.asarray(inp["Wc4"], np.float16),
        "bc1_rep": _rep(inp["bc1"]), "bc2_rep": _rep(inp["bc2"]),
        "bc3_rep": _rep(inp["bc3"]), "bc4_rep": _rep(inp["bc4"]),
        "xbioT": xbT_sel, "Qb1": q1pk, "Qb2": q2,
        "Wb1": np.asarray(inp["Wb1"], np.float16),
        "Wb2": np.asarray(inp["Wb2"], np.float16),
        "bb1_rep": _rep(inp["bb1"]),
        "bb2_row": np.ascontiguousarray(
            np.asarray(inp["bb2"], np.float32).reshape(1, -1)),
        "x_mol": np.asarray(inp["x_mol"], np.float32),
        "xmolT": np.ascontiguousarray(inp["x_mol"].T.astype(np.float32)),
        "mol_idx": mol_idx, "mol_slot": mol_slot,
        "Wm1r": np.asarray(inp["Wm1r"], np.float32),
        "Wm1s": np.asarray(inp["Wm1s"], np.float32),
        "Wm2r": np.asarray(inp["Wm2r"], np.float32),
        "Wm2s": np.asarray(inp["Wm2s"], np.float32),
        "bm1_rep": _rep(inp["bm1"]), "bm2_rep": _rep(inp["bm2"]),
        "Wlm": np.asarray(inp["Wlm"], np.float32), "blm_col": _col(inp["blm"]),
        "Wlb": np.asarray(inp["Wlb"], np.float32), "blb_col": _col(inp["blb"]),
        "Wd1": np.asarray(inp["Wd1"], np.float32),
        "bd1_t": _btile(inp["bd1"], 125, 4),
        "Wd2": np.asarray(inp["Wd2"], np.float32),
        "bd2_t": _btile(inp["bd2"], 128, 2),
        "Wcat1": np.asarray(inp["Wcat1"], np.float16),
        "bcat1_t": _btile(inp["bcat1"], 125, 8),
        "Wcat2": np.asarray(inp["Wcat2"], np.float32),
        "bcat2_t": np.asarray(inp["bcat2"], np.float32).reshape(1, 1),
        "bl1c_t": _btile(inp["bl1c"], 125, 8),
        "Wl2c": np.asarray(inp["Wl2c"], np.float16),
        "bl2c_t": _btile(inp["bl2c"], 125, 8),
        "Wl3c": np.asarray(inp["Wl3c"], np.float16),
        "bl3c_t": _btile(inp["bl3c"], 128, 2),
        "iota32": iota, "ident32": ident,
        "ones32": np.ones((P, 1), np.float32),
    }
    in_maps = []
    for c in range(NCORES):
        m = dict(shared)
        lo = c * CLL_NPC
        m["xcllT"] = np.ascontiguousarray(
            xcT[:, lo:lo + CLL_NPC]).astype(np.float16)
        # Qt packed [128, 27*4*128]: [p, (s*4+b)*128+d] = q[s*128+p, lo+b*128+d]
        qc = q[:CLL_NCH * P, lo:lo + CLL_NPC]
        m["Qt"] = np.ascontiguousarray(
            qc.reshape(CLL_NCH, P, CLL_NBLK, P).transpose(1, 0, 2, 3)
            .reshape(P, CLL_NCH * CLL_NBLK * P)).astype(np.float16)
        # W1ct [128, 12*1000]: [p, (ch*4+blk)*1000+q] = w1c_n[lo+blk*128+p, ch, q]
        wslice = w1c_n[lo:lo + CLL_NPC]                         # [512, 3, 1000]
        m["W1ct"] = np.ascontiguousarray(
            wslice.reshape(CLL_NBLK, P, 3, 1000).transpose(1, 2, 0, 3)
            .reshape(P, 12 * 1000)).astype(np.float16)
        in_maps.append(m)
    return in_maps, meta


# ------------------------------------------------------------ device program

RG = [list(range(NCORES))]


def _declare_inputs(nc):
    spec = {
        "xcllT": ([512, CLL_NPC], f16),
        "Qt": ([P, CLL_NCH * CLL_NBLK * P], f16),
        "Wc1": ([512, F], f16), "Wc2": ([F, F], f16), "Wc3": ([F, F], f16),
        "Wc4": ([F, 3], f16),
        "bc1_rep": ([P, F], f32), "bc2_rep": ([P, F], f32),
        "bc3_rep": ([P, F], f32), "bc4_rep": ([P, 3], f32),
        "W1ct": ([P, 12 * 1000], f16),
        "xbioT": ([256, BIO_S2], f16),
        "Qb1": ([P, BIO_NCH * BIO_S1], f16), "Qb2": ([BIO_S1, 1], f16),
        "Wb1": ([256, F], f16), "Wb2": ([F, F], f16),
        "bb1_rep": ([P, F], f32), "bb2_row": ([1, F], f32),
        "x_mol": ([MOL_N, 64], f32), "xmolT": ([64, MOL_N], f32),
        "mol_idx": ([P, 8], i16), "mol_slot": ([P, 1], f32),
        "Wm1r": ([64, F], f32), "Wm1s": ([64, F], f32),
        "Wm2r": ([F, F], f32), "Wm2s": ([F, F], f32),
        "bm1_rep": ([P, F], f32), "bm2_rep": ([P, F], f32),
        "Wlm": ([F, 128], f32), "blm_col": ([128, 1], f32),
        "Wlb": ([F, 128], f32), "blb_col": ([128, 1], f32),
        "Wd1": ([256, 500], f32), "bd1_t": ([125, 4], f32),
        "Wd2": ([500, 256], f32), "bd2_t": ([128, 2], f32),
        "Wcat1": ([512, 1000], f16), "bcat1_t": ([125, 8], f32),
        "Wcat2": ([1000, 1], f32), "bcat2_t": ([1, 1], f32),
        "bl1c_t": ([125, 8], f32),
        "Wl2c": ([1000, 1000], f16), "bl2c_t": ([125, 8], f32),
        "Wl3c": ([1000, 256], f16), "bl3c_t": ([128, 2], f32),
        "iota32": ([P, P], f32), "ident32": ([P, P], f32),
        "ones32": ([P, 1], f32),
    }
    return {k: nc.dram_tensor(k, s, d, kind="ExternalInput")
            for k, (s, d) in spec.items()}


def build_program(meta=None, repeat=1):
    nc = bacc.Bacc("TRN2", target_bir_lowering=False, debug=False,
                   enable_asserts=False, num_devices=NCORES,
                   num_swdge_queues=4)
    io = _declare_inputs(nc)
    out = nc.dram_tensor("out", [1, 1], f32, kind="ExternalOutput")

    h_slice = [nc.dram_tensor(f"h{l}_slice", [CLL_NPC, F], f16,
                              kind="Internal") for l in range(4)]
    h_full = [nc.dram_tensor(f"h{l}_full", [CLL_PAD, F], f16,
                             kind="Internal", addr_space="Shared")
              for l in range(4)]
    m1_dram = nc.dram_tensor("m1_dram", [MOL_N, 256], f32, kind="Internal")
    ar_in = nc.dram_tensor("ar_in", [1000], f32, kind="Internal")
    ar_out = nc.dram_tensor("ar_out", [1000], f32, kind="Internal",
                            addr_space="Shared")

    with tile.TileContext(nc) as tc:
        for _ in range(repeat):
            _build(nc, tc, io, out, h_slice, h_full, m1_dram, ar_in, ar_out)
    nc.compile()
    return nc


def _build(nc, tc, io, out, h_slice, h_full, m1_dram, ar_in, ar_out):
    with (
        tc.tile_pool(name="const", bufs=1) as cp,
        tc.tile_pool(name="wp", bufs=1) as wp,
        tc.tile_pool(name="hp", bufs=2) as hp,
        tc.tile_pool(name="sb", bufs=3) as sb,
        tc.tile_pool(name="ct", bufs=2) as ctp,
        tc.tile_pool(name="psA", bufs=2, space="PSUM") as psA,
        tc.tile_pool(name="psT", bufs=2, space="PSUM") as psT,
        tc.tile_pool(name="psM", bufs=2, space="PSUM") as psM,
    ):
        def load(pool, name, rows=None, cols=None, tag=None, dt=None):
            src = io[name]
            r = rows if rows is not None else src.shape[0]
            c = cols if cols is not None else src.shape[1]
            t = pool.tile([r, c], dt or src.dtype, tag=tag or name)
            nc.sync.dma_start(t[:], src[0:r, 0:c])
            return t

        def load2(name, tag, rows=F, cols=F):
            """[rows>128, cols] -> two tiles [128, cols] + [rows-128, cols]."""
            a = load(wp, name, rows=P, cols=cols, tag=tag + "a")
            b = wp.tile([P, cols], io[name].dtype, tag=tag + "b")
            nc.sync.dma_start(b[0:rows - P, :], io[name][P:rows, 0:cols])
            return a, b

        iota32 = load(cp, "iota32")
        ident32 = load(cp, "ident32")
        ones32 = load(cp, "ones32")

        def loadrows(name, nparts, cols, tag, rows=P):
            """Tall [nparts*rows?, cols] tensor -> list of [128, cols] tiles."""
            ts = []
            for k in range(nparts):
                t = wp.tile([rows, cols], io[name].dtype, tag=f"{tag}{k}")
                nc.sync.dma_start(t[:], io[name][k * rows:(k + 1) * rows,
                                                 0:cols])
                ts.append(t)
            return ts

        # ---- phase A: h1 = x_cll @ Wc1 (own slice), then AG1 ----
        xc = loadrows("xcllT", 4, CLL_NPC, "xc")   # 4x [128, 512] f16
        wc1 = loadrows("Wc1", 4, F, "wc1")         # 4x [128, 200] f16
        for b in range(CLL_NBLK):
            ps = psM.tile([P, F], f32, tag="m", space="PSUM")
            for k in range(4):
                nc.tensor.matmul(ps[:], xc[k][:, b * P:(b + 1) * P],
                                 wc1[k][:],
                                 start=(k == 0), stop=(k == 3))
            hst = sb.tile([P, F], f16, tag="hst")
            nc.vector.tensor_copy(hst[:], ps[:])
            nc.sync.dma_start(h_slice[0][b * P:(b + 1) * P, 0:F], hst[:])

        def allgather(l):
            nc.gpsimd.collective_compute(
                "AllGather", mybir.AluOpType.bypass, replica_groups=RG,
                ins=[h_slice[l].ap()], outs=[h_full[l].ap()])

        allgather(0)

        # ---- weight/const loads that can stream during AG1 ----
        qt = load(wp, "Qt")                        # [128, 13824] f16
        wc2 = load2("Wc2", "wc2")
        wc3 = load2("Wc3", "wc3")
        wc4 = load2("Wc4", "wc4", cols=3)
        bc_rep = [load(wp, f"bc{l}_rep") for l in (1, 2, 3)]
        bc4r = load(wp, "bc4_rep")
        w1ct = load(wp, "W1ct")                    # [128, 12000] f16

        def hload(l):
            t = hp.tile([P, CLL_NCH, F], f16, tag="hft")
            nc.sync.dma_start(
                t[:], h_full[l].ap().rearrange("(c p) f -> p c f", p=P)[:, 0:CLL_NCH, :])
            return t

        def transpose_to(src_sb, dst0, dst1, bcol):
            """src [128, 200] f32 -> dst0[128, bcol:+128], dst1[72, bcol:+128] f16."""
            pt = psT.tile([P, P], f32, tag="tp", space="PSUM")
            nc.tensor.transpose(pt[0:P, 0:P], src_sb[:, 0:P], ident32[:])
            nc.vector.tensor_copy(dst0[:, bcol:bcol + P], pt[0:P, 0:P])
            pt2 = psT.tile([P, P], f32, tag="tp", space="PSUM")
            nc.tensor.transpose(pt2[0:F - P, 0:P], src_sb[:, P:F], ident32[:])
            nc.vector.tensor_copy(dst1[0:F - P, bcol:bcol + P],
                                  pt2[0:F - P, 0:P])

        def cll_layer(l, hft, wnext, brep):
            """Aggregate layer l from table hft; produce h_slice[l+1] (if
            wnext) by per-block transform; returns list of c blocks (f32)."""
            cb = []
            cT0 = ctp.tile([P, CLL_NPC], f16, tag="cT0")
            cT1 = ctp.tile([P, CLL_NPC], f16, tag="cT1")
            for b in range(CLL_NBLK):
                ps = psA.tile([P, F], f32, tag="agg", space="PSUM")
                for s in range(CLL_NCH):
                    qcol = (s * CLL_NBLK + b) * P
                    nc.tensor.matmul(ps[:], qt[:, qcol:qcol + P],
                                     hft[:, s, 0:F],
                                     start=(s == 0), stop=(s == CLL_NCH - 1))
                t2 = sb.tile([P, F], f32, tag="ev1")
                nc.vector.tensor_tensor(t2[:], ps[:], brep[:], op=ADD)
                cblk = sb.tile([P, F], f32, tag="cblk", bufs=4)
                nc.scalar.activation(cblk[:], t2[:], RELU)
                cb.append(cblk)
                transpose_to(cblk, cT0, cT1, b * P)
                if wnext is not None:
                    wa, wb_ = wnext
                    ph = psM.tile([P, F], f32, tag="m", space="PSUM")
                    nc.tensor.matmul(ph[:], cT0[:, b * P:(b + 1) * P], wa[:],
                                     start=True, stop=False)
                    nc.tensor.matmul(ph[:], cT1[0:F - P, b * P:(b + 1) * P],
                                     wb_[0:F - P, :], start=False, stop=True)
                    hst = sb.tile([P, F], f16, tag="hst")
                    nc.vector.tensor_copy(hst[:], ph[:])
                    nc.sync.dma_start(h_slice[l + 1][b * P:(b + 1) * P, 0:F],
                                      hst[:])
                else:
                    hst = sb.tile([P, F], f16, tag="hst")
                    nc.vector.tensor_copy(hst[:], cblk[:])
                    nc.sync.dma_start(h_slice[l + 1][b * P:(b + 1) * P, 0:F],
                                      hst[:])
            return cb

        # ---- layer 1 ----
        hft = hload(0)
        cll_layer(0, hft, wc2, bc_rep[0])
        allgather(1)

        # ---- bio mini-branch (runs while AG2 is in flight) ----
        xbT = loadrows("xbioT", 2, BIO_S2, "xbT")  # 2x [128, 768] f16
        qb1 = load(wp, "Qb1")                      # [128, 768] f16
        qb2 = load(wp, "Qb2")                      # [128, 1] f16
        wb1 = loadrows("Wb1", 2, F, "wb1")         # 2x [128, 200] f16
        wb2 = load2("Wb2", "wb2")
        bb1r = load(wp, "bb1_rep")
        bb2row = load(wp, "bb2_row")
        h1b = sb.tile([P, BIO_NCH, F], f16, tag="h1b", bufs=1)
        for j in range(BIO_NCH):
            ps = psM.tile([P, F], f32, tag="m", space="PSUM")
            for k in range(2):
                nc.tensor.matmul(ps[:], xbT[k][:, j * P:(j + 1) * P],
                                 wb1[k][:],
                                 start=(k == 0), stop=(k == 1))
            nc.vector.tensor_copy(h1b[:, j, :], ps[:])
        psb = psM.tile([P, F], f32, tag="m", space="PSUM")
        for j in range(BIO_NCH):
            nc.tensor.matmul(psb[:], qb1[:, j * P:(j + 1) * P], h1b[:, j, 0:F],
                             start=(j == 0), stop=(j == BIO_NCH - 1))
        tb1 = sb.tile([P, F], f32, tag="ev1")
        nc.vector.tensor_tensor(tb1[:], psb[:], bb1r[:], op=ADD)
        c1b = sb.tile([P, F], f32, tag="c1b", bufs=1)
        nc.scalar.activation(c1b[:], tb1[:], RELU)
        c1bT0 = sb.tile([P, P], f16, tag="c1bT0", bufs=1)
        c1bT1 = sb.tile([P, P], f16, tag="c1bT1", bufs=1)
        ptb = psT.tile([P, P], f32, tag="tp", space="PSUM")
        nc.tensor.transpose(ptb[0:P, 0:P], c1b[:, 0:P], ident32[:])
        nc.vector.tensor_copy(c1bT0[:], ptb[0:P, 0:P])
        ptb2 = psT.tile([P, P], f32, tag="tp", space="PSUM")
        nc.tensor.transpose(ptb2[0:F - P, 0:P], c1b[:, P:F], ident32[:])
        nc.vector.tensor_copy(c1bT1[0:F - P, :], ptb2[0:F - P, 0:P])
        ph2 = psM.tile([P, F], f32, tag="m", space="PSUM")
        nc.tensor.matmul(ph2[:], c1bT0[:, 0:P], wb2[0][:],
                         start=True, stop=False)
        nc.tensor.matmul(ph2[:], c1bT1[0:F - P, 0:P], wb2[1][0:F - P, :],
                         start=False, stop=True)
        h2b = sb.tile([P, F], f16, tag="h2b", bufs=1)
        nc.vector.tensor_copy(h2b[:], ph2[:])
        pr = psM.tile([1, F], f32, tag="m", space="PSUM")
        nc.tensor.matmul(pr[:], qb2[:], h2b[:], start=True, stop=True)
        tb2 = sb.tile([1, F], f32, tag="ev1")
        nc.vector.tensor_tensor(tb2[0:1, :], pr[0:1, :], bb2row[0:1, :], op=ADD)
        brow = sb.tile([1, F], f32, tag="brow", bufs=1)
        nc.scalar.activation(brow[0:1, :], tb2[0:1, :], RELU)
        # bvec column [200, 1] for the head
        bgc0 = sb.tile([P, 1], f32, tag="bgc0", bufs=1)
        bgc1 = sb.tile([P, 1], f32, tag="bgc1", bufs=1)
        prc = psT.tile([P, P], f32, tag="tp", space="PSUM")
        nc.tensor.transpose(prc[0:P, 0:1], brow[0:1, 0:P], ident32[0:1, 0:1])
        nc.vector.tensor_copy(bgc0[:], prc[0:P, 0:1])
        prc2 = psT.tile([P, P], f32, tag="tp", space="PSUM")
        nc.tensor.transpose(prc2[0:F - P, 0:1], brow[0:1, P:F],
                            ident32[0:1, 0:1])
        nc.vector.tensor_copy(bgc1[0:F - P, :], prc2[0:F - P, 0:1])

        # ---- layer 2 ----
        hft = hload(1)
        cll_layer(1, hft, wc3, bc_rep[1])
        allgather(2)

        # ---- mol branch (runs while AG3 is in flight) ----
        mol_idx_sb = load(cp, "mol_idx")
        mol_slot_sb = load(cp, "mol_slot")
        xmolT_sb = load(wp, "xmolT")
        wm1r = load(wp, "Wm1r")
        wm1s = load(wp, "Wm1s")
        bm1r = load(wp, "bm1_rep")
        bm2r = load(wp, "bm2_rep")
        v1 = sb.tile([P, 1, 64], f32, tag="vm")
        nc.gpsimd.dma_gather(v1[:], io["x_mol"].ap(), mol_idx_sb[:],
                             MOL_E, MOL_E, 64)
        mM = sb.tile([P, 64], f32, tag="Mmol", bufs=1)
        nc.vector.tensor_scalar(mM[:], iota32[:, 0:64], mol_slot_sb[:, 0:1],
                                None, op0=EQ)
        agg_ps = psM.tile([64, 64], f32, tag="m", space="PSUM")
        nc.tensor.matmul(agg_ps[:], mM[:], v1[:, 0, :], start=True, stop=True)
        agg_sb = sb.tile([64, 64], f32, tag="mol1")
        nc.vector.tensor_copy(agg_sb[:], agg_ps[:])
        pt = psT.tile([P, P], f32, tag="tp", space="PSUM")
        nc.tensor.transpose(pt[0:64, 0:64], agg_sb[0:64, 0:64],
                            ident32[0:64, 0:64])
        aggT = sb.tile([64, 64], f32, tag="mol2")
        nc.vector.tensor_copy(aggT[:], pt[0:64, 0:64])
        h1_ps = psM.tile([64, F], f32, tag="m", space="PSUM")
        nc.tensor.matmul(h1_ps[:], aggT[:], wm1r[:], start=True, stop=False)
        nc.tensor.matmul(h1_ps[:], xmolT_sb[:], wm1s[:], start=False, stop=True)
        t_m1 = sb.tile([64, F], f32, tag="mol3")
        nc.vector.tensor_tensor(t_m1[:], h1_ps[:], bm1r[0:64, :], op=ADD)
        m1_sb = sb.tile([64, F], f32, tag="mol4", bufs=1)
        nc.scalar.activation(m1_sb[:], t_m1[:], RELU)
        nc.sync.dma_start(m1_dram[0:64, 0:F], m1_sb[:])

        wm2r = load2("Wm2r", "wm2r")
        wm2s = load2("Wm2s", "wm2s")
        v2 = sb.tile([P, 1, 256], f32, tag="vm2")
        nc.gpsimd.dma_gather(v2[:], m1_dram.ap(), mol_idx_sb[:],
                             MOL_E, MOL_E, 256)
        agg2_ps = psM.tile([64, F], f32, tag="m", space="PSUM")
        nc.tensor.matmul(agg2_ps[:], mM[:], v2[:, 0, 0:F], start=True, stop=True)
        agg2_sb = sb.tile([64, F], f32, tag="mol1")
        nc.vector.tensor_copy(agg2_sb[:], agg2_ps[:])
        a2T0 = sb.tile([P, 64], f32, tag="mol5")
        a2T1 = sb.tile([P, 64], f32, tag="mol6")
        m1T0 = sb.tile([P, 64], f32, tag="mol7")
        m1T1 = sb.tile([P, 64], f32, tag="mol8")
        for srcT, d0, d1 in ((agg2_sb, a2T0, a2T1), (m1_sb, m1T0, m1T1)):
            pt1 = psT.tile([P, P], f32, tag="tp", space="PSUM")
            nc.tensor.transpose(pt1[0:P, 0:64], srcT[0:64, 0:P],
                                ident32[0:64, 0:64])
            nc.vector.tensor_copy(d0[:, 0:64], pt1[0:P, 0:64])
            pt2 = psT.tile([P, P], f32, tag="tp", space="PSUM")
            nc.tensor.transpose(pt2[0:F - P, 0:64], srcT[0:64, P:F],
                                ident32[0:64, 0:64])
            nc.vector.tensor_copy(d1[0:F - P, 0:64], pt2[0:F - P, 0:64])
        h2_ps = psM.tile([64, F], f32, tag="m", space="PSUM")
        nc.tensor.matmul(h2_ps[:], a2T0[:, 0:64], wm2r[0][:],
                         start=True, stop=False)
        nc.tensor.matmul(h2_ps[:], a2T1[0:F - P, 0:64], wm2r[1][0:F - P, :],
                         start=False, stop=False)
        nc.tensor.matmul(h2_ps[:], m1T0[:, 0:64], wm2s[0][:],
                         start=False, stop=False)
        nc.tensor.matmul(h2_ps[:], m1T1[0:F - P, 0:64], wm2s[1][0:F - P, :],
                         start=False, stop=True)
        t_m2 = sb.tile([64, F], f32, tag="mol3")
        nc.vector.tensor_tensor(t_m2[:], h2_ps[:], bm2r[0:64, :], op=ADD)
        m2_sb = sb.tile([64, F], f32, tag="mol4", bufs=1)
        nc.scalar.activation(m2_sb[:], t_m2[:], RELU)

        wlm = load2("Wlm", "wlm", cols=128)
        blm = load(wp, "blm_col")
        mcol0 = sb.tile([P, 1], f32, tag="mc0", bufs=1)
        mcol1 = sb.tile([P, 1], f32, tag="mc1", bufs=1)
        pool_ps = psM.tile([P, 1], f32, tag="m", space="PSUM")
        nc.tensor.matmul(pool_ps[0:P, :], m2_sb[0:64, 0:P], ones32[0:64, :],
                         start=True, stop=True)
        nc.scalar.activation(mcol0[:], pool_ps[0:P, :], COPY, scale=1.0 / 64.0)
        pool_ps2 = psM.tile([P, 1], f32, tag="m", space="PSUM")
        nc.tensor.matmul(pool_ps2[0:F - P, :], m2_sb[0:64, P:F],
                         ones32[0:64, :], start=True, stop=True)
        nc.scalar.activation(mcol1[0:F - P, :], pool_ps2[0:F - P, :], COPY,
                             scale=1.0 / 64.0)
        mvec = sb.tile([P, 1], f32, tag="mvec", bufs=1)
        mm_ps = psM.tile([P, 1], f32, tag="m", space="PSUM")
        nc.tensor.matmul(mm_ps[:], wlm[0][:], mcol0[:], start=True, stop=False)
        nc.tensor.matmul(mm_ps[:], wlm[1][0:F - P, :], mcol1[0:F - P, :],
                         start=False, stop=True)
        nc.scalar.activation(mvec[:], mm_ps[:], RELU, bias=blm[:])

        # ---- layer 3 (produces c3 slices -> AG4) ----
        hft = hload(2)
        cll_layer(2, hft, None, bc_rep[2])
        allgather(3)

        # ---- head weight loads (stream during AG4) ----
        wlb = load2("Wlb", "wlb", cols=128)
        blb = load(wp, "blb_col")
        wd1 = load2("Wd1", "wd1", rows=256, cols=500)
        bd1 = load(wp, "bd1_t")
        wd2t = [wp.tile([125, 256], f32, tag=f"wd2_{k}", name=f"wd2_{k}")
                for k in range(4)]
        for k in range(4):
            nc.sync.dma_start(wd2t[k][:], io["Wd2"][k * 125:(k + 1) * 125, :])
        bd2 = load(wp, "bd2_t")
        bl1c = load(wp, "bl1c_t")
        bl2c = load(wp, "bl2c_t")
        wtc2 = [wp.tile([125, 1000], f16, tag=f"wl2c_{k}", name=f"wl2c_{k}")
                for k in range(8)]
        for k in range(8):
            nc.sync.dma_start(wtc2[k][:], io["Wl2c"][k * 125:(k + 1) * 125, :])
        bl3c = load(wp, "bl3c_t")
        wtc3 = [wp.tile([125, 256], f16, tag=f"wl3c_{k}", name=f"wl3c_{k}")
                for k in range(8)]
        for k in range(8):
            nc.sync.dma_start(wtc3[k][:], io["Wl3c"][k * 125:(k + 1) * 125, :])
        bcat1 = load(wp, "bcat1_t")
        wtu = [wp.tile([P, 1000], f16, tag=f"wcat1_{k}", name=f"wcat1_{k}")
               for k in range(4)]
        for k in range(4):
            nc.sync.dma_start(wtu[k][:], io["Wcat1"][k * P:(k + 1) * P, :])
        wcat2 = wp.tile([125, 8], f32, tag="wcat2")
        for k in range(8):
            nc.sync.dma_start(wcat2[:, k:k + 1],
                              io["Wcat2"][k * 125:(k + 1) * 125, 0:1])
        bcat2 = load(wp, "bcat2_t")

        # ---- layer 4: aggregate c3, transform by Wc4, dense-1 partials ----
        hft = hload(3)
        h4pack = sb.tile([P, 12], f16, tag="h4p", bufs=1)
        for b in range(CLL_NBLK):
            ps = psA.tile([P, F], f32, tag="agg", space="PSUM")
            for s in range(CLL_NCH):
                qcol = (s * CLL_NBLK + b) * P
                nc.tensor.matmul(ps[:], qt[:, qcol:qcol + P], hft[:, s, 0:F],
                                 start=(s == 0), stop=(s == CLL_NCH - 1))
            ag = sb.tile([P, F], f32, tag="ev1")
            nc.vector.tensor_copy(ag[:], ps[:])
            aT0 = sb.tile([P, P], f16, tag="a4T0")
            aT1 = sb.tile([P, P], f16, tag="a4T1")
            pt4 = psT.tile([P, P], f32, tag="tp", space="PSUM")
            nc.tensor.transpose(pt4[0:P, 0:P], ag[:, 0:P], ident32[:])
            nc.vector.tensor_copy(aT0[:], pt4[0:P, 0:P])
            pt5 = psT.tile([P, P], f32, tag="tp", space="PSUM")
            nc.tensor.transpose(pt5[0:F - P, 0:P], ag[:, P:F], ident32[:])
            nc.vector.tensor_copy(aT1[0:F - P, :], pt5[0:F - P, 0:P])
            ph4 = psM.tile([P, 3], f32, tag="m", space="PSUM")
            nc.tensor.matmul(ph4[:], aT0[:, 0:P], wc4[0][:],
                             start=True, stop=False)
            nc.tensor.matmul(ph4[:], aT1[0:F - P, 0:P], wc4[1][0:F - P, :],
                             start=False, stop=True)
            th4 = sb.tile([P, 3], f32, tag="th4")
            nc.vector.tensor_tensor(th4[:], ph4[:], bc4r[:, 0:3], op=ADD)
            h4b = sb.tile([P, 3], f32, tag="h4b")
            nc.scalar.activation(h4b[:], th4[:], RELU)
            for ch in range(3):
                nc.vector.tensor_copy(h4pack[:, ch * 4 + b:ch * 4 + b + 1],
                                      h4b[:, ch:ch + 1])

        dsum = sb.tile([1, 1000], f32, tag="dsum", bufs=1)
        for half in range(2):
            psd = psM.tile([1, 500], f32, tag="m", space="PSUM")
            for j in range(12):
                nc.tensor.matmul(psd[:], h4pack[:, j:j + 1],
                                 w1ct[:, j * 1000 + half * 500:
                                      j * 1000 + half * 500 + 500],
                                 start=(j == 0), stop=(j == 11))
            nc.vector.tensor_copy(dsum[0:1, half * 500:half * 500 + 500],
                                  psd[0:1, :])
        nc.sync.dma_start(ar_in.ap()[0:1000, None], dsum[0:1, :])

        nc.gpsimd.collective_compute(
            "AllReduce", mybir.AluOpType.add, replica_groups=RG,
            ins=[ar_in.ap()], outs=[ar_out.ap()])

        # ---- fusion head (replicated) ----
        def mm_chain(p_rows, n_cols, k_steps, act_bias, out_tag):
            acc = sb.tile([p_rows, n_cols], f32, tag=out_tag + "a")
            for k in range(k_steps):
                lhsT, rhs = yield k
                pst = psM.tile([p_rows, n_cols], f32, tag="m", space="PSUM")
                for och in range(n_cols):
                    nc.tensor.matmul(pst[:, och:och + 1], lhsT(och), rhs,
                                     start=True, stop=True)
                if k == 0:
                    nc.vector.tensor_copy(acc[:], pst[:])
                else:
                    nc.vector.tensor_tensor(acc[:], acc[:], pst[:], op=ADD)
            o = sb.tile([p_rows, n_cols], f32, tag=out_tag, bufs=1)
            for och in range(n_cols):
                nc.scalar.activation(o[:, och:och + 1], acc[:, och:och + 1],
                                     RELU, bias=act_bias[:, och:och + 1])
            yield o

        def run_chain(p_rows, n_cols, pieces, act_bias, out_tag):
            gen = mm_chain(p_rows, n_cols, len(pieces), act_bias, out_tag)
            k = next(gen)
            while True:
                r = gen.send(pieces[k])
                if not isinstance(r, int):
                    return r
                k = r

        bvec = run_chain(P, 1, [
            (lambda o: wlb[0][:, 0:128], bgc0[:]),
            (lambda o: wlb[1][0:F - P, 0:128], bgc1[0:F - P, :]),
        ], blb, "bvec")

        d1 = run_chain(125, 4, [
            (lambda o: wd1[0][:, o * 125:(o + 1) * 125], mvec[:]),
            (lambda o: wd1[1][:, o * 125:(o + 1) * 125], bvec[:]),
        ], bd1, "d1")

        d2 = run_chain(P, 2, [
            (lambda o, k=k: wd2t[k][:, o * P:(o + 1) * P], d1[:, k:k + 1])
            for k in range(4)
        ], bd2, "d2")

        c1 = sb.tile([125, 8], f32, tag="c1", bufs=1)
        for j in range(8):
            tmpc = sb.tile([125, 1], f32, tag="ctmp")
            nc.sync.dma_start(tmpc[:], ar_out.ap()[j * 125:(j + 1) * 125, None])
            nc.scalar.activation(c1[:, j:j + 1], tmpc[:], RELU,
                                 bias=bl1c[:, j:j + 1])
        c1h = sb.tile([125, 8], f16, tag="c1h", bufs=1)
        nc.vector.tensor_copy(c1h[:], c1[:])

        c2 = run_chain(125, 8, [
            (lambda o, k=k: wtc2[k][:, o * 125:(o + 1) * 125], c1h[:, k:k + 1])
            for k in range(8)
        ], bl2c, "c2")
        c2h = sb.tile([125, 8], f16, tag="c2h", bufs=1)
        nc.vector.tensor_copy(c2h[:], c2[:])

        c3 = run_chain(P, 2, [
            (lambda o, k=k: wtc3[k][:, o * P:(o + 1) * P], c2h[:, k:k + 1])
            for k in range(8)
        ], bl3c, "c3")

        cat_h = sb.tile([P, 4], f16, tag="cath", bufs=1)
        nc.vector.tensor_copy(cat_h[:, 0:1], d2[:, 0:1])
        nc.vector.tensor_copy(cat_h[:, 1:2], d2[:, 1:2])
        nc.vector.tensor_copy(cat_h[:, 2:3], c3[:, 0:1])
        nc.vector.tensor_copy(cat_h[:, 3:4], c3[:, 1:2])
        u = run_chain(125, 8, [
            (lambda o, k=k: wtu[k][:, o * 125:(o + 1) * 125], cat_h[:, k:k + 1])
            for k in range(4)
        ], bcat1, "u")

        pso = psM.tile([1, 1], f32, tag="m", space="PSUM")
        for k in range(8):
            nc.tensor.matmul(pso[:], wcat2[:, k:k + 1], u[:, k:k + 1],
                             start=(k == 0), stop=(k == 7))
        osb = sb.tile([1, 1], f32, tag="osb", bufs=1)
        nc.scalar.activation(osb[:], pso[:], RELU, bias=bcat2[:])
        nc.sync.dma_start(out[0:1, 0:1], osb[:])


# ------------------------------------------------------------------- entry

_CACHE = {}


def kernel(**inputs):
    in_maps, meta = prep_inputs(inputs)
    if "nc" not in _CACHE:
        _CACHE["nc"] = build_program(meta)
    nc = _CACHE["nc"]
    res = run_bass_kernel_spmd(nc, in_maps, core_ids=list(range(NCORES)))
    return np.asarray(res.results[0]["out"], np.float32)
